# revision 22
# baseline (speedup 1.0000x reference)
"""Multi-head attention kernel for Trainium2, 8 NeuronCores, data-parallel over batch.

Problem: batch=16, pos=577, d_model=1024, n_heads=16, d_head=64, fp32.
Sharding: batch across 8 cores (2 batch items per core), no collectives.

v3: restructured for PE issue-rate roofline (measured: N/2.4GHz per matmul
only when consecutive matmuls hit different PSUM banks and LDWEIGHTS hides):
  - all accumulation chains alternate PSUM banks between consecutive matmuls
  - score matmuls for the two heads of a pair are emitted adjacently with
    row-disjoint tile_position -> they run concurrently in the PE array
  - q-chunks (512, 65): exp reads contiguous 4-bank PSUM gulps on ACT
    (352-cycle fixed overhead amortized over 2048 elems)
  - augmented V = [1 | V_h]: AV matmul row 0 = softmax denominator;
    reciprocal reads the PSUM row directly; DRAM-roundtrip broadcast
  - fine-grained emission interleave: projection chunk-units fill the
    PE queue while ACT exp gulps drain score banks
"""
import numpy as np

import concourse.bass as bass
import concourse.tile as tile
from concourse import bacc, mybir

F32 = mybir.dt.float32
BF16 = mybir.dt.bfloat16
AF = mybir.ActivationFunctionType

NCORES = 8
B = 2            # batch per core
T = 577
D = 1024
H = 16
E = 64
HE = H * E       # 1024
BT = B * T       # 1154
MT = 8           # m-tiles over HE (head pairs)
KT = 8           # k-tiles over D
VW = E + 1       # 65: augmented V width per head [1 | V]

A_CH = [(0, 385), (385, 385), (770, 384)]                       # proj chunks over BT
TT = [(0, 128), (128, 128), (256, 128), (384, 128), (512, 65)]  # tiles over T (keys)
QN = [(0, 512), (512, 65)]                                      # q chunks
MO = [(0, 128), (128, 128), (256, 128), (384, 128), (512, 65)]  # out-proj m tiles
# qi=1 score tiles: tile j=(2*kt+hd) -> psS bank j%3, col (j//3)*65
# pp column base per gulp, within one (b,hp): see ppcol()


def ppcol(qi, kt, hd):
    if qi == 0:
        return kt * 1024 + hd * 512
    j = 2 * kt + hd
    return 5120 + (j % 2) * 512 + (j // 2) * 65


PP_W = 5120 + 1024  # 6144 bf16 cols per (b,hp)


def build_graph():
    nc = bacc.Bacc("TRN2", target_bir_lowering=False, debug=False,
                   num_devices=NCORES)

    xq = nc.dram_tensor("query_input", [D, BT], BF16, kind="ExternalInput")
    xk = nc.dram_tensor("key_input", [D, BT], BF16, kind="ExternalInput")
    xv = nc.dram_tensor("value_input", [D, BT], BF16, kind="ExternalInput")
    wq = nc.dram_tensor("W_Q", [D, HE], BF16, kind="ExternalInput")
    wk = nc.dram_tensor("W_K", [D, HE], BF16, kind="ExternalInput")
    wv = nc.dram_tensor("W_V", [D, HE], BF16, kind="ExternalInput")
    wo = nc.dram_tensor("W_O", [HE, D], BF16, kind="ExternalInput")
    bq = nc.dram_tensor("b_Q", [128, MT], F32, kind="ExternalInput")
    bk = nc.dram_tensor("b_K", [128, MT], F32, kind="ExternalInput")
    bv = nc.dram_tensor("b_V", [128, MT], F32, kind="ExternalInput")
    bo = nc.dram_tensor("b_O", [1, D], F32, kind="ExternalInput")
    out = nc.dram_tensor("out", [B, T, D], F32, kind="ExternalOutput")

    with tile.TileContext(nc) as tc:
        _body(nc, tc, xq, xk, xv, wq, wk, wv, wo, bq, bk, bv, bo, out)
    nc.compile()
    return nc


def _body(nc, tc, xq, xk, xv, wq, wk, wv, wo, bq, bk, bv, bo, out):
    from contextlib import ExitStack
    est = ExitStack()
    with est:
        sbQ_p = est.enter_context(tc.tile_pool(name="sbQ", bufs=1))
        sbK_p = est.enter_context(tc.tile_pool(name="sbK", bufs=1))
        sbVg_p = est.enter_context(tc.tile_pool(name="sbVg", bufs=1))
        sbZ_p = est.enter_context(tc.tile_pool(name="sbZ", bufs=1))
        xt_p = est.enter_context(tc.tile_pool(name="xt", bufs=3))
        wt_p = est.enter_context(tc.tile_pool(name="wt", bufs=3))
        pp_p = est.enter_context(tc.tile_pool(name="pp", bufs=2))
        dn_p = est.enter_context(tc.tile_pool(name="dn", bufs=2))
        const_p = est.enter_context(tc.tile_pool(name="const", bufs=1))
        dram_p = est.enter_context(tc.tile_pool(name="dramd", bufs=1, space="DRAM"))

        bqc = const_p.tile([128, MT], F32, tag="bqc")
        bkc = const_p.tile([128, MT], F32, tag="bkc")
        bvc = const_p.tile([128, MT], F32, tag="bvc")
        boc = const_p.tile([128, D], F32, tag="boc")
        nc.sync.dma_start(bqc[:], bq.ap())
        nc.sync.dma_start(bkc[:], bk.ap())
        nc.sync.dma_start(bvc[:], bv.ap())
        nc.sync.dma_start(boc[:], bo.ap().partition_broadcast(128))

        # rings over head-pair m: slot = m % 3
        sbQ = sbQ_p.tile([128, 3 * BT], BF16, tag="sbQ")
        sbK = sbK_p.tile([128, 3 * BT], BF16, tag="sbK")
        sbVg = sbVg_p.tile([128, 10 * H * VW], BF16, tag="sbVg")
        sbZ = sbZ_p.tile([128, B * MT * T], BF16, tag="sbZ")

        def zsl(b, hp, lo, sz, to, tsz):
            base = (b * MT + hp) * T
            return sbZ[lo:lo + sz, base + to:base + to + tsz]

        def load_xw(x_in, w_in, xtag, wtag):
            xt = xt_p.tile([128, KT * BT], BF16, tag="xt", name=xtag)
            wt = wt_p.tile([128, KT * HE], BF16, tag="wt", name=wtag)
            for k in range(KT):
                nc.sync.dma_start(xt[:, k * BT:(k + 1) * BT],
                                  x_in.ap()[k * 128:(k + 1) * 128, :])
                nc.scalar.dma_start(wt[:, k * HE:(k + 1) * HE],
                                    w_in.ap()[k * 128:(k + 1) * 128, :])
            return xt, wt

        # ================= Phase V (runs first, alone) =================
        with tc.tile_pool(name="psV", bufs=4, space="PSUM") as psV_p:
            xtv = xt_p.tile([128, KT * BT], BF16, tag="xt", name="xtv")
            wtv = wt_p.tile([128, KT * HE], BF16, tag="wt", name="wtv")
            # k0 split fine so the first matmul's DMA dependency is small
            nc.sync.dma_start(xtv[:, 0:128], xv.ap()[0:128, 0:128])
            nc.scalar.dma_start(wtv[:, 0:512], wv.ap()[0:128, 0:512])
            nc.sync.dma_start(xtv[:, 128:BT], xv.ap()[0:128, 128:BT])
            nc.scalar.dma_start(wtv[:, 512:HE], wv.ap()[0:128, 512:HE])
            for k in range(1, KT):
                nc.sync.dma_start(xtv[:, k * BT:(k + 1) * BT],
                                  xv.ap()[k * 128:(k + 1) * 128, :])
                nc.scalar.dma_start(wtv[:, k * HE:(k + 1) * HE],
                                    wv.ap()[k * 128:(k + 1) * 128, :])
            for b in range(B):
                for ti, (to, tsz) in enumerate(TT):
                    vt = b * 5 + ti
                    vbase = vt * H * VW
                    bto = b * T + to
                    pss = [psV_p.tile([128, 512], F32, tag="psV",
                                      name=f"psV{vt}_{ni}") for ni in range(2)]
                    for k in range(KT):
                        for ni in range(2):
                            nc.tensor.matmul(
                                pss[ni][:tsz, :],
                                xtv[:, k * BT + bto:k * BT + bto + tsz],
                                wtv[:, k * HE + ni * 512:k * HE + ni * 512 + 512],
                                start=(k == 0), stop=(k == KT - 1))
                    for ni in range(2):
                        # strided eviction: 8 heads at once, V at [1:65] per head
                        dst = sbVg[:tsz, vbase + ni * 8 * VW:
                                   vbase + (ni * 8 + 8) * VW].rearrange(
                            "p (h c) -> p h c", c=VW)[:, :, 0:E]
                        src = pss[ni][:tsz, :].rearrange(
                            "p (h c) -> p h c", c=E)
                        nc.vector.tensor_copy(dst, src)
                    onecols = sbVg[:tsz, vbase:vbase + H * VW].rearrange(
                        "p (h c) -> p h c", c=VW)[:, :, E:E + 1]
                    nc.vector.memset(onecols, 1.0)

        xtq, wtq = load_xw(xq, wq, "xtq", "wtq")
        xtk, wtk = load_xw(xk, wk, "xtk", "wtk")

        # ================= m-loop: A (Q/K proj) interleaved with B ========
        bs = ExitStack()
        psS_p = bs.enter_context(tc.tile_pool(name="psS", bufs=2, space="PSUM"))
        psZ_p = bs.enter_context(tc.tile_pool(name="psZ", bufs=2, space="PSUM"))
        pa = ExitStack()
        psA_p = pa.enter_context(tc.tile_pool(name="psA", bufs=2, space="PSUM"))

        # ---- A units ----
        a_queue = []

        def make_a_units(m):
            # 6 chunks: (proj, chunk) pairs -> 3 pairs -> 6 half-units
            chunks = [(xtq, wtq, bqc, sbQ, ci) for ci in range(3)] + \
                     [(xtk, wtk, bkc, sbK, ci) for ci in range(3)]
            for pi in range(3):
                pair = chunks[2 * pi:2 * pi + 2]
                tiles = [psA_p.tile([128, 385], F32, tag="psA",
                                    name=f"psA_m{m}_{pi}_{j}")
                         for j in range(2)]

                def quarter(h, pair=pair, tiles=tiles, m=m):
                    for k in range(2 * h, 2 * h + 2):
                        for j, (xt, wt, bc, dst, ci) in enumerate(pair):
                            co, csz = A_CH[ci]
                            nc.tensor.matmul(
                                tiles[j][:, :csz],
                                wt[:, k * HE + m * 128:k * HE + (m + 1) * 128],
                                xt[:, k * BT + co:k * BT + co + csz],
                                start=(k == 0), stop=(k == KT - 1))
                    if h == 3:
                        r = m % 3
                        for j, (xt, wt, bc, dst, ci) in enumerate(pair):
                            co, csz = A_CH[ci]
                            nc.vector.tensor_scalar_add(
                                dst[:, r * BT + co:r * BT + co + csz],
                                tiles[j][:, :csz], bc[:, m:m + 1])

                for h in range(4):
                    a_queue.append(lambda fn=quarter, h=h: fn(h))

        def fill(n=1):
            for _ in range(n):
                if a_queue:
                    a_queue.pop(0)()

        def make_filler(budget):
            box = [budget]

            def bf(n=1):
                take = min(n, box[0], len(a_queue))
                box[0] -= take
                fill(take)
            return bf

        # ---- B emission for one (b, hp) ----
        def emit_bhp(b, hp, fills=fill):
            r = hp % 3
            qb = b * T
            pp = pp_p.tile([128, PP_W], BF16, tag="pp", name=f"pp{b}_{hp}")
            ddf = dn_p.tile([33, T], F32, tag="ddf", name=f"ddf{b}_{hp}")
            ddb = dn_p.tile([33, T], BF16, tag="ddb", name=f"ddb{b}_{hp}")
            psz = {}

            def sc(qi, kt):
                # one fresh 2-bank psS tile per (qi0, kt); one tile for all qi1
                st = psS_p.tile([128, 1024], F32, tag="psS",
                                name=f"psS{b}_{hp}_{qi}_{kt}")
                qo, qsz = QN[qi]
                if qi == 0:
                    ko, ksz = TT[kt]
                    for hd in range(2):
                        lo = hd * 64
                        nc.tensor.matmul(
                            st[:ksz, hd * 512:hd * 512 + qsz],
                            sbK[lo:lo + 64, r * BT + qb + ko:r * BT + qb + ko + ksz],
                            sbQ[lo:lo + 64, r * BT + qb + qo:r * BT + qb + qo + qsz],
                            start=True, stop=True, tile_position=(lo, 0))
                else:
                    for kt2 in range(5):
                        ko, ksz = TT[kt2]
                        for hd in range(2):
                            lo = hd * 64
                            j = 2 * kt2 + hd
                            dcol = (j % 2) * 512 + (j // 2) * 65
                            nc.tensor.matmul(
                                st[:ksz, dcol:dcol + qsz],
                                sbK[lo:lo + 64, r * BT + qb + ko:r * BT + qb + ko + ksz],
                                sbQ[lo:lo + 64, r * BT + qb + qo:r * BT + qb + qo + qsz],
                                start=True, stop=True, tile_position=(lo, 0))
                # flat full-tile exp gulp: write-once-read-once per generation
                rows = 128 if qi == 1 or TT[kt][1] == 128 else 65
                base = kt * 1024 if qi == 0 else 5120
                nc.scalar.activation(pp[:rows, base:base + 1024],
                                     st[:rows, :], AF.Exp, scale=0.125)

            def av(qi, kts):
                qo, qsz = QN[qi]
                for kt in kts:
                    ko, ksz = TT[kt]
                    vbase = (b * 5 + kt) * H * VW
                    for hd in range(2):
                        h = 2 * hp + hd
                        if kt == 0:
                            psz[(qi, hd)] = psZ_p.tile(
                                [65, 512], F32, tag="psZ",
                                name=f"psZ{b}_{hp}_{qi}_{hd}")
                        nc.tensor.matmul(
                            psz[(qi, hd)][:, :qsz],
                            sbVg[:ksz, vbase + h * VW:vbase + h * VW + VW],
                            pp[:ksz, ppcol(qi, kt, hd):ppcol(qi, kt, hd) + qsz],
                            start=(kt == 0), stop=(kt == 4))

            def finz(qi):
                qo, qsz = QN[qi]
                for hd in range(2):
                    lo = hd * 64
                    nc.vector.tensor_scalar_add(
                        zsl(b, hp, lo, 64, qo, qsz),
                        psz[(qi, hd)][0:64, :qsz],
                        bvc[lo:lo + 64, hp:hp + 1])
                    nc.vector.tensor_copy(
                        ddf[32 * hd:32 * hd + 1, qo:qo + qsz],
                        psz[(qi, hd)][64:65, :qsz])

            # ---- emission sequence (ACT-paced: one fill per kt) ----
            sc(0, 0)
            sc(0, 1)
            fills()
            av(0, [0])
            sc(0, 2)
            fills()
            av(0, [1])
            sc(0, 3)
            fills()
            av(0, [2])
            sc(0, 4)
            fills()
            av(0, [3])
            av(0, [4])
            finz(0)
            sc(1, 0)
            fills()
            av(1, range(5))
            finz(1)
            # normalize: broadcast 1/denom via DRAM roundtrip, then scale z
            nc.vector.reciprocal_approx_fast(ddf[:], ddf[:])
            nc.vector.tensor_copy(ddb[:], ddf[:])
            dd = dram_p.tile([2, T], BF16, tag=f"dd{b}_{hp}", name=f"dd{b}_{hp}")
            rb = dn_p.tile([128, T], BF16, tag="rb", name=f"rb{b}_{hp}")
            for hd in range(2):
                lo = hd * 64
                nc.sync.dma_start(dd[hd:hd + 1, :], ddb[32 * hd:32 * hd + 1, :])
                nc.sync.dma_start(
                    rb[lo:lo + 64, :],
                    dd[hd:hd + 1, :].partition_broadcast(64))
                nc.gpsimd.tensor_mul(zsl(b, hp, lo, 64, 0, T),
                                     zsl(b, hp, lo, 64, 0, T), rb[lo:lo + 64, :])
            fills()

        # ---- the m-loop ----
        make_a_units(0)
        fill(len(a_queue))  # prime ALL of m0: B(0,0) must not pull its own deps
        # W_O prefetch: reuses wv's buffer slot, DMA starts once V-phase is done
        wot = wt_p.tile([128, MT * D], BF16, tag="wt", name="wot")
        for hp in range(MT):
            nc.scalar.dma_start(wot[:, hp * D:(hp + 1) * D],
                                wo.ap()[hp * 128:(hp + 1) * 128, :])
        for m in range(1, MT):
            make_a_units(m)
            emit_bhp(0, m - 1, fills=make_filler(6))
            emit_bhp(1, m - 1, fills=make_filler(6))
            fill(len(a_queue))  # drain leftovers to stay ahead
        # tail: last head pair; phase C interleaves below
        # ================= Phase C =================
        pa.close()  # frees psA (2 banks) before psO opens

        sbO_p = est.enter_context(tc.tile_pool(name="sbO", bufs=3))
        co = ExitStack()
        psO_p = co.enter_context(tc.tile_pool(name="psO", bufs=2, space="PSUM"))

        c_open = []

        def c_prefix(b, mi, pool=None):
            pool = pool or psO_p
            mo, msz = MO[mi]
            tiles = [pool.tile([128, 512], F32, tag="psO",
                               name=f"psO{b}_{mi}_{ni}") for ni in range(2)]
            for hp in range(MT - 1):
                for ni in range(2):
                    nc.tensor.matmul(
                        tiles[ni][:msz, :],
                        zsl(b, hp, 0, 128, mo, msz),
                        wot[:, hp * D + ni * 512:hp * D + ni * 512 + 512],
                        start=(hp == 0), stop=False)
            c_open.append((b, mi, tiles))

        def c_finish():
            b, mi, tiles = c_open.pop(0)
            mo, msz = MO[mi]
            hp = MT - 1
            for ni in range(2):
                nc.tensor.matmul(
                    tiles[ni][:msz, :],
                    zsl(b, hp, 0, 128, mo, msz),
                    wot[:, hp * D + ni * 512:hp * D + ni * 512 + 512],
                    start=False, stop=True)
            for ni in range(2):
                so = sbO_p.tile([128, 512], F32, tag="sbO",
                                name=f"sbO{b}_{mi}_{ni}")
                nc.vector.tensor_add(so[:msz, :], tiles[ni][:msz, :],
                                     boc[:msz, ni * 512:ni * 512 + 512])
                eng = nc.sync if ni == 0 else nc.scalar
                eng.dma_start(
                    out.ap()[b, mo:mo + msz, ni * 512:ni * 512 + 512],
                    so[:msz, :])

        c_queue = [(b, mi) for b in range(B) for mi in range(5)]

        def cfill_b0(n=1):
            # during emit_bhp(0,7): only hp0-6 prefixes of b0 units; max 2 open
            for _ in range(n):
                if c_queue and c_queue[0][0] == 0 and len(c_open) < 2:
                    b, mi = c_queue.pop(0)
                    c_prefix(b, mi)

        def cfill_b1(n=1):
            # during emit_bhp(1,7): z(b0) fully ready -> finish opens, run b0 fully
            for _ in range(n):
                if c_open:
                    c_finish()
                elif c_queue and c_queue[0][0] == 0:
                    b, mi = c_queue.pop(0)
                    c_prefix(b, mi)

        emit_bhp(0, MT - 1, fills=cfill_b0)
        while c_open:
            c_finish()
        emit_bhp(1, MT - 1, fills=cfill_b1)
        while c_open:
            c_finish()
        co.close()
        bs.close()
        psO3_p = est.enter_context(tc.tile_pool(name="psO3", bufs=4, space="PSUM"))
        while c_queue:
            b, mi = c_queue.pop(0)
            c_prefix(b, mi, pool=psO3_p)
            c_finish()


_GRAPH = None


def _get_graph():
    global _GRAPH
    if _GRAPH is None:
        _GRAPH = build_graph()
    return _GRAPH


def kernel(query_input, key_input, value_input, W_Q, W_K, W_V, W_O,
           b_Q, b_K, b_V, b_O, _trace=False, _trace_kwargs=None):
    import ml_dtypes
    from concourse.bass_utils import run_bass_kernel_spmd

    nc = _get_graph()
    f = np.ascontiguousarray
    bf = ml_dtypes.bfloat16

    def xT(x, sl):
        x = np.asarray(x[sl], np.float32)
        return f(x.reshape(B * T, D).T.astype(bf))

    def wT(w):
        w = np.asarray(w, np.float32)
        return f(w.transpose(1, 0, 2).reshape(D, HE).astype(bf))

    def bcol(bx):
        bx = np.asarray(bx, np.float32).reshape(HE)
        return f(bx.reshape(MT, 128).T)

    wq_m, wk_m, wv_m = wT(W_Q), wT(W_K), wT(W_V)
    wo_m = f(np.asarray(W_O, np.float32).reshape(HE, D).astype(bf))
    bq_m, bk_m, bv_m = bcol(b_Q), bcol(b_K), bcol(b_V)
    bo_m = f(np.asarray(b_O, np.float32).reshape(1, D))
    in_maps = []
    for c in range(NCORES):
        sl = slice(2 * c, 2 * c + 2)
        in_maps.append({
            "query_input": xT(query_input, sl),
            "key_input": xT(key_input, sl),
            "value_input": xT(value_input, sl),
            "W_Q": wq_m,
            "W_K": wk_m,
            "W_V": wv_m,
            "W_O": wo_m,
            "b_Q": bq_m,
            "b_K": bk_m,
            "b_V": bv_m,
            "b_O": bo_m,
        })
    res = run_bass_kernel_spmd(nc, in_maps, core_ids=list(range(NCORES)),
                               trace=_trace, **(_trace_kwargs or {}))
    outp = np.concatenate([res.results[c]["out"] for c in range(NCORES)], axis=0)
    if _trace:
        kernel._last_result = res
    return outp
